# revision 2
# baseline (speedup 1.0000x reference)
"""Trainium2 Bass kernel for nn_DCTHighPass.

Reference computation (per sample, 512x512 RGB image):
  gray = 0.299 R + 0.587 G + 0.114 B
  tiles = 8x8 blocks of gray (64x64 tiles, row-major (ti, tj))
  mag = |fft2(tile)|
  (buggy mask touches only batch 3:6 / fft rows 3:6 -> never sampled below)
  img = mag tiles stacked into [4096*8, 8]
  out = bilinear_resize(img, 512, 512)

Key reduction: the height downsample (32768 -> 512, factor 64) samples only
input rows 64i+31 and 64i+32 with weight 0.5 each, i.e. fft-row 7 of tile
(ti=i//8, tj=8*(i%8)+3) and fft-row 0 of tile (ti, tj=8*(i%8)+4).  fft rows
0/7 of an 8x8 real tile need only three 8-weight row reductions of the tile
(plain sum, cos, sin), followed by an 8-point DFT along columns.  The width
upsample (8 -> 512) is a fixed [8,512] matrix.  So per output row i:
  v = 0.5*(|DFT(cos/sin rowsums of tile tj3)| + |DFT(colsum of tile tj4)|)
  out[i, :] = v @ W8
Only image columns 64p+24 .. 64p+39 (p = i%8) are ever used.

Pipeline per sample (per core, batch of 8 samples):
  stage1 (PE):  x[128-row chunk] as stationary operand (128 needed cols),
                weights = per-channel (gray-coef x {1,cos,sin} x row-group)
                -> PSUM [128=(p,cc), 192=(q,type,tI_l)]
  stage2 (PE):  4 block-diag DFT matmuls -> [64=(p,k), 192] cos/sin x g0/g1
  magnitude (ACT/DVE): sqrt of sum of squares, x0.5 -> V [64=(p,k), 64=tI]
  stage3 (PE):  V (stationary) @ p-masked replicated W8 -> [64=tI, 512]
  out rows i = 8*tI + p, interleaved stores back to HBM.
"""

import sys

sys.path.insert(0, "/opt/trn_rl_repo")

import math
import numpy as np

from concourse import bacc
import concourse.mybir as mybir
from concourse.tile import TileContext
from concourse.bass_utils import run_bass_kernel_spmd

N_CORES = 8
B_FULL = 64
B_CORE = B_FULL // N_CORES  # 8 samples per core
H = W = 512
K = 8  # fft tile size
NQ = 4  # 128-row chunks per image
DT = mybir.dt.float32


# ----------------------------------------------------------------------------
# host-side constants
# ----------------------------------------------------------------------------
def _make_constants():
    j = np.arange(K)
    cosr = np.cos(2 * np.pi * j / K)
    sinr = np.sin(2 * np.pi * j / K)

    # wred [128, 144]: free = 48*ch + 16*type + tI_l
    # type 0: plain sum (A), 1: cos rowsum (Cr), 2: sin rowsum (Ci)
    coef = [0.299, 0.587, 0.114]
    wtypes = [np.ones(K), cosr, sinr]
    wred = np.zeros((128, 3 * 48), dtype=np.float32)
    r = np.arange(128)
    for ch in range(3):
        for ty in range(3):
            for t in range(16):
                rows = slice(8 * t, 8 * t + 8)
                wred[rows, 48 * ch + 16 * ty + t] = coef[ch] * wtypes[ty]

    # dft matrices C[v,c] = cos(2pi v c/8), S[v,c] = sin(2pi v c/8)
    v = np.arange(K)
    C8 = np.cos(2 * np.pi * np.outer(v, j) / K).astype(np.float32)
    S8 = np.sin(2 * np.pi * np.outer(v, j) / K).astype(np.float32)

    # dftc [128, 256]: 4 blocks of 64 cols: [C*g0 | S*g0 | C*g1 | S*g1]
    # partition = 16p + cc (cc in 0..15, g = cc//8); out col = 64*s + 8p + k
    dftc = np.zeros((128, 256), dtype=np.float32)
    for p in range(8):
        for cc in range(16):
            g, c = divmod(cc, 8)
            for k in range(8):
                cv, sv = C8[k, c], S8[k, c]
                if g == 0:
                    dftc[16 * p + cc, 0 + 8 * p + k] = cv
                    dftc[16 * p + cc, 64 + 8 * p + k] = sv
                else:
                    dftc[16 * p + cc, 128 + 8 * p + k] = cv
                    dftc[16 * p + cc, 192 + 8 * p + k] = sv

    # W8 [8, 512]: bilinear width resize 8 -> 512 (align_corners=False)
    src = (np.arange(W) + 0.5) * (K / W) - 0.5
    src = np.clip(src, 0.0, K - 1.0)
    i0 = np.floor(src).astype(np.int64)
    i1 = np.minimum(i0 + 1, K - 1)
    fr = (src - i0).astype(np.float32)
    W8 = np.zeros((K, W), dtype=np.float32)
    for jj in range(W):
        W8[i0[jj], jj] += 1.0 - fr[jj]
        W8[i1[jj], jj] += fr[jj]

    # wrep [64, 8*512]: block p holds W8 on partitions 8p..8p+7, zero elsewhere
    wrep = np.zeros((64, 8 * W), dtype=np.float32)
    for p in range(8):
        wrep[8 * p : 8 * p + 8, W * p : W * p + W] = W8

    return wred, dftc, wrep


_WRED, _DFTC, _WREP = _make_constants()


# ----------------------------------------------------------------------------
# bass program (identical on all cores; per-core inputs differ)
# ----------------------------------------------------------------------------
def _build_program():
    nc = bacc.Bacc()

    xs = nc.declare_dram_parameter("xs", [B_CORE, 3, H, W], DT, isOutput=False)
    wred_d = nc.declare_dram_parameter("wred", [128, 144], DT, isOutput=False)
    dftc_d = nc.declare_dram_parameter("dftc", [128, 256], DT, isOutput=False)
    wrep_d = nc.declare_dram_parameter("wrep", [64, 8 * W], DT, isOutput=False)
    ys = nc.declare_dram_parameter("ys", [B_CORE, 1, H, W], DT, isOutput=True)

    with TileContext(nc) as tc:
        with (
            tc.tile_pool(name="consts", bufs=1) as cpool,
            tc.tile_pool(name="xin", bufs=2) as xpool,
            tc.tile_pool(name="mid", bufs=2) as mpool,
            tc.tile_pool(name="outp", bufs=2) as opool,
            tc.tile_pool(name="ps1", bufs=2, space="PSUM") as ps1pool,
            tc.tile_pool(name="ps2", bufs=1, space="PSUM") as ps2pool,
            tc.tile_pool(name="ps3", bufs=2, space="PSUM") as ps3pool,
        ):
            wred_sb = cpool.tile([128, 144], DT, tag="wred")
            nc.sync.dma_start(wred_sb[:], wred_d[:])
            dftc_sb = cpool.tile([128, 256], DT, tag="dftc")
            nc.sync.dma_start(dftc_sb[:], dftc_d[:])
            wrep_sb = cpool.tile([64, 8 * W], DT, tag="wrep")
            nc.sync.dma_start(wrep_sb[:], wrep_d[:])

            for bg in range(B_CORE):
                # ---- load sample: [128, (ch, q, c)] = 3 MB ----
                xa = xpool.tile([128, 3 * NQ * W], DT, tag="xa")
                nc.sync.dma_start(
                    xa.rearrange("p (ch q c) -> p ch q c", ch=3, q=NQ),
                    xs[bg].rearrange("ch (q p) c -> p ch q c", p=128),
                )
                xav = xa.rearrange("p (ch q g c) -> p ch q g c", ch=3, q=NQ, g=8)

                # gather the 128 needed columns per (ch, q) contiguously:
                # matmul stationary operands must have a single free dim
                xneed = xpool.tile([128, 3 * NQ * 128], DT, tag="xneed")
                xneedv = xneed.rearrange("p (ch q g c) -> p ch q g c", ch=3, q=NQ, g=8)
                for ch in range(3):
                    eng = nc.vector if ch != 1 else nc.scalar
                    if eng is nc.vector:
                        eng.tensor_copy(xneedv[:, ch], xav[:, ch, :, :, 24:40])
                    else:
                        eng.copy(xneedv[:, ch], xav[:, ch, :, :, 24:40])
                xneedf = xneed.rearrange("p (blk c) -> p blk c", c=128)

                # ---- stage 1: row reductions, out [128=(p,cc), 192=(q,ty,tI_l)]
                ps1 = ps1pool.tile([128, 192], DT, tag="ps1")
                for q in range(NQ):
                    for ch in range(3):
                        nc.tensor.matmul(
                            ps1[:, 48 * q : 48 * q + 48],
                            xneedf[:, NQ * ch + q],  # [128, 128] needed cols
                            wred_sb[:, 48 * ch : 48 * ch + 48],
                            start=(ch == 0),
                            stop=(ch == 2),
                        )
                rhs2 = mpool.tile([128, 192], DT, tag="rhs2")
                nc.vector.tensor_copy(rhs2[:], ps1[:])

                # ---- stage 2: DFT matmuls -> [64=(p,k), 192] ----
                psA = ps2pool.tile([64, 192], DT, tag="psA")  # C * g0
                psB = ps2pool.tile([64, 192], DT, tag="psB")  # S * g0
                psC = ps2pool.tile([64, 192], DT, tag="psC")  # C * g1
                psD = ps2pool.tile([64, 192], DT, tag="psD")  # S * g1
                for s, pt in enumerate([psA, psB, psC, psD]):
                    nc.tensor.matmul(
                        pt[:], dftc_sb[:, 64 * s : 64 * s + 64], rhs2[:],
                        start=True, stop=True,
                    )

                # free-dim selectors: type block ty at offset 16*ty, q stride 48
                def sel(ps, ty):
                    return ps.rearrange("p (q blk) -> p q blk", q=NQ)[
                        :, :, 16 * ty : 16 * ty + 16
                    ]

                sbB = mpool.tile([64, 192], DT, tag="sbB")
                nc.scalar.copy(sbB[:], psB[:])
                sbBv = sbB.rearrange("p (q blk) -> p q blk", q=NQ)

                def t2(t):
                    return t.rearrange("p (q w) -> p q w", q=NQ)

                # m3: fft row 7 of tile tj=8p+3   (types: 1=Cr, 2=Ci)
                r3 = mpool.tile([64, 64], DT, tag="r3")
                nc.vector.tensor_add(t2(r3), sel(psA, 1), sbBv[:, :, 32:48])
                i3 = mpool.tile([64, 64], DT, tag="i3")
                nc.vector.tensor_sub(t2(i3), sel(psA, 2), sbBv[:, :, 16:32])
                p3 = mpool.tile([64, 64], DT, tag="p3")
                nc.scalar.activation(p3[:], r3[:], mybir.ActivationFunctionType.Square)
                q3 = mpool.tile([64, 64], DT, tag="q3")
                nc.scalar.activation(q3[:], i3[:], mybir.ActivationFunctionType.Square)
                s3 = mpool.tile([64, 64], DT, tag="s3")
                nc.vector.tensor_add(s3[:], p3[:], q3[:])
                m3 = mpool.tile([64, 64], DT, tag="m3")
                nc.scalar.activation(
                    m3[:], s3[:], mybir.ActivationFunctionType.Sqrt, scale=0.25
                )

                # m4: fft row 0 of tile tj=8p+4   (type 0 = plain sum)
                p4 = mpool.tile([64, 64], DT, tag="p4")
                nc.scalar.activation(
                    t2(p4), sel(psC, 0), mybir.ActivationFunctionType.Square
                )
                q4 = mpool.tile([64, 64], DT, tag="q4")
                nc.scalar.activation(
                    t2(q4), sel(psD, 0), mybir.ActivationFunctionType.Square
                )
                s4 = mpool.tile([64, 64], DT, tag="s4")
                nc.vector.tensor_add(s4[:], p4[:], q4[:])
                m4 = mpool.tile([64, 64], DT, tag="m4")
                nc.scalar.activation(
                    m4[:], s4[:], mybir.ActivationFunctionType.Sqrt, scale=0.25
                )

                vt = mpool.tile([64, 64], DT, tag="vt")
                nc.vector.tensor_add(vt[:], m3[:], m4[:])

                # ---- stage 3: width resize, rows i = 8*tI + p ----
                outsb = opool.tile([128, NQ * W], DT, tag="outsb")
                for v in range(4):
                    ps3 = ps3pool.tile([128, W], DT, tag="ps3")
                    for e in range(2):
                        p = 2 * v + e
                        nc.tensor.matmul(
                            ps3[64 * e : 64 * e + 64, :],
                            vt[:],
                            wrep_sb[:, W * p : W * p + W],
                            start=True, stop=True,
                        )
                    eng = nc.vector if (v % 2 == 0) else nc.scalar
                    if eng is nc.vector:
                        eng.tensor_copy(outsb[:, W * v : W * v + W], ps3[:])
                    else:
                        eng.copy(outsb[:, W * v : W * v + W], ps3[:])

                nc.sync.dma_start(
                    ys[bg, 0].rearrange("(t v e) j -> e t v j", v=4, e=2),
                    outsb.rearrange("p (v j) -> p v j", v=4),
                )

    nc.compile()
    return nc


_NC = None


def _get_program():
    global _NC
    if _NC is None:
        _NC = _build_program()
    return _NC


def kernel(x: np.ndarray) -> np.ndarray:
    assert x.shape == (B_FULL, 3, H, W), x.shape
    x = np.ascontiguousarray(x, dtype=np.float32)
    nc = _get_program()
    in_maps = []
    for c in range(N_CORES):
        in_maps.append(
            {
                "xs": x[c * B_CORE : (c + 1) * B_CORE],
                "wred": _WRED,
                "dftc": _DFTC,
                "wrep": _WREP,
            }
        )
    res = run_bass_kernel_spmd(nc, in_maps, core_ids=list(range(N_CORES)))
    out = np.concatenate([res.results[c]["ys"] for c in range(N_CORES)], axis=0)
    return out


# revision 30
# speedup vs baseline: 42168.5796x; 42168.5796x over previous
"""Trainium2 Bass kernel for nn_DCTHighPass.

Reference computation (per sample, 512x512 RGB image):
  gray = 0.299 R + 0.587 G + 0.114 B
  tiles = 8x8 blocks of gray (64x64 tiles, row-major (ti, tj))
  mag = |fft2(tile)|
  (buggy mask touches only batch 3:6 / fft rows 3:6 -> never sampled below)
  img = mag tiles stacked into [4096*8, 8]
  out = bilinear_resize(img, 512, 512)

Key reduction: the height downsample (32768 -> 512, factor 64) samples only
input rows 64i+31 and 64i+32 with weight 0.5 each, i.e. fft-row 7 of tile
(ti=i//8, tj=8*(i%8)+3) and fft-row 0 of tile (ti, tj=8*(i%8)+4).  fft rows
0/7 of an 8x8 real tile need only three 8-weight row reductions of the tile
(plain sum, cos, sin), followed by an 8-point DFT along columns.  The width
upsample (8 -> 512) is a fixed [8,512] matrix.  So per output row i:
  v = 0.5*(|DFT(cos/sin rowsums of tile tj3)| + |DFT(colsum of tile tj4)|)
  out[i, :] = v @ W8
Only image columns 64p+24 .. 64p+39 (p = i%8) are ever used.

Pipeline per sample (per core, batch of 8 samples):
  stage1 (PE):  x[128-row chunk] as stationary operand (128 needed cols),
                weights = per-channel (gray-coef x {1,cos,sin} x row-group)
                -> PSUM [128=(p,cc), 192=(q,type,tI_l)]
  stage2 (PE):  4 block-diag DFT matmuls -> [64=(p,k), 192] cos/sin x g0/g1
  magnitude (ACT/DVE): sqrt of sum of squares, x0.5 -> V [64=(p,k), 64=tI]
  stage3 (PE):  V (stationary) @ p-masked replicated W8 -> [64=tI, 512]
  out rows i = 8*tI + p, interleaved stores back to HBM.
"""

import sys

sys.path.insert(0, "/opt/trn_rl_repo")

import math
import numpy as np

from concourse import bacc
import concourse.mybir as mybir
from concourse.tile import TileContext
from concourse.bass_utils import run_bass_kernel_spmd

N_CORES = 8
B_FULL = 64
B_CORE = B_FULL // N_CORES  # 8 samples per core
H = W = 512
K = 8  # fft tile size
NQ = 4  # 128-row chunks per image
DT = mybir.dt.float32


# ----------------------------------------------------------------------------
# host-side constants
# ----------------------------------------------------------------------------
def _make_constants():
    j = np.arange(K)
    cosr = np.cos(2 * np.pi * j / K)
    sinr = np.sin(2 * np.pi * j / K)

    # wred [128, 144]: free = 48*ch + 16*type + tI_l
    # type 0: plain sum (A), 1: cos rowsum (Cr), 2: sin rowsum (Ci)
    coef = [0.299, 0.587, 0.114]
    wtypes = [np.ones(K), cosr, sinr]
    wred = np.zeros((128, 4 * 48), dtype=np.float32)
    r = np.arange(128)
    for ch in range(3):
        for ty in range(3):
            for t in range(16):
                rows = slice(8 * t, 8 * t + 8)
                wred[rows, 48 * ch + 16 * ty + t] = coef[ch] * wtypes[ty]
    for ty in range(3):
        for t in range(16):
            rows = slice(8 * t, 8 * t + 8)
            wred[rows, 144 + 16 * ty + t] = wtypes[ty]

    # dft matrices C[v,c] = cos(2pi v c/8), S[v,c] = sin(2pi v c/8)
    v = np.arange(K)
    C8 = np.cos(2 * np.pi * np.outer(v, j) / K).astype(np.float32)
    S8 = np.sin(2 * np.pi * np.outer(v, j) / K).astype(np.float32)

    # dftc [128, 320]: 5 blocks of 64 cols: [C*g0 | S*g0 | -S*g0 | C*g1 | S*g1]
    # partition = 16p + cc (cc in 0..15, g = cc//8); out col = 64*s + 8p + k
    dftc = np.zeros((128, 320), dtype=np.float32)
    for p in range(8):
        for cc in range(16):
            g, c = divmod(cc, 8)
            for k in range(8):
                cv, sv = C8[k, c], S8[k, c]
                if g == 0:
                    dftc[16 * p + cc, 0 + 8 * p + k] = cv
                    dftc[16 * p + cc, 64 + 8 * p + k] = sv
                    dftc[16 * p + cc, 128 + 8 * p + k] = -sv
                else:
                    dftc[16 * p + cc, 192 + 8 * p + k] = cv
                    dftc[16 * p + cc, 256 + 8 * p + k] = sv

    # W8 [8, 512]: bilinear width resize 8 -> 512 (align_corners=False)
    src = (np.arange(W) + 0.5) * (K / W) - 0.5
    src = np.clip(src, 0.0, K - 1.0)
    i0 = np.floor(src).astype(np.int64)
    i1 = np.minimum(i0 + 1, K - 1)
    fr = (src - i0).astype(np.float32)
    W8 = np.zeros((K, W), dtype=np.float32)
    for jj in range(W):
        W8[i0[jj], jj] += 1.0 - fr[jj]
        W8[i1[jj], jj] += fr[jj]

    # wrep [64, 8*512]: block p holds W8 on partitions 8p..8p+7, zero elsewhere
    wrep = np.zeros((64, 8 * W), dtype=np.float32)
    for p in range(8):
        wrep[8 * p : 8 * p + 8, W * p : W * p + W] = W8

    return wred, dftc, wrep


_WRED, _DFTC, _WREP = _make_constants()


# ----------------------------------------------------------------------------
# bass program (identical on all cores; per-core inputs differ)
# ----------------------------------------------------------------------------
CFG = dict(ps1_bufs=2, psab_bufs=2, ps3_bufs=4, load_split=2, store_split=1,
           copy_pat="avva", xin_bufs=3, mid_bufs=2, wrep_pool=True,
           merge_stores=True, load_mode="sparse", loads_only=False, skip_stores=False,
           gray_pre=True)


def _build_program(repeat=1):
    nc = bacc.Bacc()

    xs = nc.declare_dram_parameter("xs", [B_CORE, 3, H, W], DT, isOutput=False)
    wred_d = nc.declare_dram_parameter("wred", [128, 192], DT, isOutput=False)
    dftc_d = nc.declare_dram_parameter("dftc", [128, 320], DT, isOutput=False)
    wrep_d = nc.declare_dram_parameter("wrep", [64, 8 * W], mybir.dt.float32r, isOutput=False)
    ys = nc.declare_dram_parameter("ys", [B_CORE, 1, H, W], DT, isOutput=True)

    with TileContext(nc) as tc:
        with (
            tc.tile_pool(name="consts", bufs=1) as cpool,
            tc.tile_pool(name="xin", bufs=CFG["xin_bufs"]) as xpool,
            tc.tile_pool(name="xa", bufs=2) as xapool,
            tc.tile_pool(name="mid", bufs=CFG["mid_bufs"]) as mpool,
            tc.tile_pool(name="outp", bufs=2) as opool,
            tc.tile_pool(name="ps1", bufs=CFG["ps1_bufs"], space="PSUM") as ps1pool,
            tc.tile_pool(name="ps2", bufs=CFG["psab_bufs"], space="PSUM") as ps2pool,
            tc.tile_pool(name="ps3", bufs=CFG["ps3_bufs"], space="PSUM") as ps3pool,
        ):
            wred_sb = cpool.tile([128, 192], DT, tag="wred")
            nc.sync.dma_start(wred_sb[:], wred_d[:])
            dftc_sb = cpool.tile([128, 320], DT, tag="dftc")
            nc.sync.dma_start(dftc_sb[:], dftc_d[:])
            wrep_sb = cpool.tile([64, 8 * W], mybir.dt.float32r, tag="wrep")
            (nc.gpsimd if CFG["wrep_pool"] else nc.sync).dma_start(
                wrep_sb[:], wrep_d[:]
            )

            rep_ctx = tc.For_i(0, repeat, 1) if repeat > 1 else None
            if rep_ctx is not None:
                rep_ctx.__enter__()
            for bg2 in range(B_CORE // 2):
                # ---- two samples per iteration: stage2/3 run at 128-wide ----
                xn = []
                li = 0
                for smp in range(2):
                    bg = 2 * bg2 + smp
                    if CFG["load_mode"] == "full":
                        # one contiguous 3MB DMA, then gather needed columns
                        xa = xapool.tile([128, 3 * NQ * W], DT, tag=f"xa{smp}")
                        nc.sync.dma_start(
                            xa.rearrange("p (ch q c) -> p ch q c", ch=3, q=NQ),
                            xs[bg].rearrange("ch (q p) c -> p ch q c", p=128),
                        )
                        xav = xa.rearrange(
                            "p (ch q g c) -> p ch q g c", ch=3, q=NQ, g=8
                        )
                        xneed = xpool.tile([128, 3 * NQ * 128], DT, tag=f"xn{smp}")
                        xneedv = xneed.rearrange(
                            "p (ch q g c) -> p ch q g c", ch=3, q=NQ, g=8
                        )
                        for ch in range(3):
                            eng = nc.vector if (ch + smp) % 2 == 0 else nc.scalar
                            if eng is nc.vector:
                                eng.tensor_copy(
                                    xneedv[:, ch], xav[:, ch, :, :, 24:40]
                                )
                            else:
                                eng.copy(xneedv[:, ch], xav[:, ch, :, :, 24:40])
                    else:
                        xneed = xpool.tile([128, 3 * NQ * 128], DT, tag=f"xn{smp}")
                        xneedv = xneed.rearrange(
                            "p (ch q g c) -> p ch q g c", ch=3, q=NQ, g=8
                        )
                        xsrc = xs[bg].rearrange(
                            "ch (q p) (g c) -> p ch q g c", p=128, g=8
                        )
                        for ch in range(3):
                            for q in range(NQ):
                                eng = (
                                    nc.scalar
                                    if (li % 6) < CFG["load_split"]
                                    else nc.sync
                                )
                                eng.dma_start(
                                    xneedv[:, ch, q], xsrc[:, ch, q, :, 24:40]
                                )
                                li += 1
                    xn.append(xneed.rearrange("p (blk c) -> p blk c", c=128))

                if CFG["loads_only"]:
                    continue
                # ---- stage 1: row reductions per sample ----
                rhs2 = mpool.tile([128, 2 * 192], DT, tag="rhs2")
                if CFG["gray_pre"]:
                    # gray = 0.299 R + 0.587 G + 0.114 B on ACT/DVE, then a
                    # single unscaled reduction matmul per (smp, q)
                    xgray = mpool.tile([128, 2 * 512], DT, tag="xgray")
                    for smp in range(2):
                        xg = xgray[:, 512 * smp : 512 * smp + 512]
                        xr = xn[smp].rearrange("p blk c -> p (blk c)")
                        t1 = mpool.tile([128, 512], DT, tag=f"t1{smp}")
                        nc.scalar.activation(
                            t1[:], xr[:, 512:1024],
                            mybir.ActivationFunctionType.Copy, scale=0.587,
                        )
                        t2 = mpool.tile([128, 512], DT, tag=f"t2{smp}")
                        nc.vector.scalar_tensor_tensor(
                            t2[:], xr[:, 0:512], 0.299, t1[:],
                            mybir.AluOpType.mult, mybir.AluOpType.add,
                        )
                        nc.vector.scalar_tensor_tensor(
                            xg, xr[:, 1024:1536], 0.114, t2[:],
                            mybir.AluOpType.mult, mybir.AluOpType.add,
                        )
                    xgv = xgray.rearrange("p (sq c) -> p sq c", c=128)
                    for smp in range(2):
                        ps1 = ps1pool.tile([128, 192], DT, tag="ps1")
                        for q in range(NQ):
                            nc.tensor.matmul(
                                ps1[:, 48 * q : 48 * q + 48],
                                xgv[:, 4 * smp + q],
                                wred_sb[:, 144:192],
                                start=True, stop=True,
                            )
                        nc.vector.tensor_copy(
                            rhs2[:, 192 * smp : 192 * smp + 192], ps1[:]
                        )
                else:
                    for smp in range(2):
                        ps1 = ps1pool.tile([128, 192], DT, tag="ps1")
                        for q in range(NQ):
                            for ch in range(3):
                                nc.tensor.matmul(
                                    ps1[:, 48 * q : 48 * q + 48],
                                    xn[smp][:, NQ * ch + q],
                                    wred_sb[:, 48 * ch : 48 * ch + 48],
                                    start=(ch == 0),
                                    stop=(ch == 2),
                                )
                        nc.vector.tensor_copy(
                            rhs2[:, 192 * smp : 192 * smp + 192], ps1[:]
                        )

                # ---- stage 2: DFT + height-blend fused via PSUM accumulation
                # psQ [64=(p,k), 512] = [R3 | I3 | R4 | I4] blocks of (smp, q, tI_l)
                rhs2v = rhs2.rearrange("p (s q blk) -> p s q blk", s=2, q=NQ)
                selA = rhs2v[:, :, :, 0:16]
                selCr = rhs2v[:, :, :, 16:32]
                selCi = rhs2v[:, :, :, 32:48]
                psQ = ps2pool.tile([64, 512], DT, tag="psQ")
                C0 = dftc_sb[:, 0:64]
                S0 = dftc_sb[:, 64:128]
                S0n = dftc_sb[:, 128:192]
                C1 = dftc_sb[:, 192:256]
                S1 = dftc_sb[:, 256:320]
                nc.tensor.matmul(psQ[:, 0:128], C0, selCr, start=True, stop=False)
                nc.tensor.matmul(psQ[:, 0:128], S0, selCi, start=False, stop=True)
                nc.tensor.matmul(psQ[:, 128:256], C0, selCi, start=True, stop=False)
                nc.tensor.matmul(psQ[:, 128:256], S0n, selCr, start=False, stop=True)
                nc.tensor.matmul(psQ[:, 256:384], C1, selA, start=True, stop=True)
                nc.tensor.matmul(psQ[:, 384:512], S1, selA, start=True, stop=True)

                # magnitudes: m = 0.5*sqrt(re^2 + im^2), [64, 128] each
                Sq = mybir.ActivationFunctionType.Square
                p3 = mpool.tile([64, 128], DT, tag="p3")
                nc.scalar.activation(p3[:], psQ[:, 0:128], Sq)
                q3 = mpool.tile([64, 128], DT, tag="q3")
                nc.scalar.activation(q3[:], psQ[:, 128:256], Sq)
                s3 = mpool.tile([64, 128], DT, tag="s3")
                nc.vector.tensor_add(s3[:], p3[:], q3[:])
                m3 = mpool.tile([64, 128], DT, tag="m3")
                nc.scalar.activation(
                    m3[:], s3[:], mybir.ActivationFunctionType.Sqrt, scale=0.25
                )
                p4 = mpool.tile([64, 128], DT, tag="p4")
                nc.scalar.activation(p4[:], psQ[:, 256:384], Sq)
                q4 = mpool.tile([64, 128], DT, tag="q4")
                nc.scalar.activation(q4[:], psQ[:, 384:512], Sq)
                s4 = mpool.tile([64, 128], DT, tag="s4")
                nc.vector.tensor_add(s4[:], p4[:], q4[:])
                m4 = mpool.tile([64, 128], DT, tag="m4")
                nc.scalar.activation(
                    m4[:], s4[:], mybir.ActivationFunctionType.Sqrt, scale=0.25
                )
                vt = mpool.tile([64, 128], mybir.dt.float32r, tag="vt")
                nc.vector.tensor_add(vt[:], m3[:], m4[:])

                # ---- stage 3: width resize; out partitions = (smp, tI) ----
                outse = opool.tile([128, NQ * W], DT, tag="outse")
                outso = opool.tile([128, NQ * W], DT, tag="outso")
                for p in range(8):
                    v, e2 = divmod(p, 2)
                    ps3 = ps3pool.tile([128, W], DT, tag="ps3")
                    nc.tensor.matmul(
                        ps3[:],
                        vt[:],
                        wrep_sb[:, W * p : W * p + W],
                        start=True, stop=True,
                    )
                    dst = (outso if e2 else outse)[:, W * v : W * v + W]
                    if CFG["copy_pat"][p % 4] == "v":
                        nc.vector.tensor_copy(dst, ps3[:])
                    else:
                        nc.scalar.copy(dst, ps3[:])

                # merged pair stores per sample: rows 8t+e and 8t+e+4
                for smp in range(2) if not CFG["skip_stores"] else []:
                    bg = 2 * bg2 + smp
                    yr2 = ys[bg, 0].rearrange(
                        "(t h e) j -> e t h j", h=2, e=4
                    )  # i = 8t + 4h + e
                    for e in range(4):
                        v0, e2 = divmod(e, 2)
                        src = outso if e2 else outse
                        sap = src.rearrange("p (u v j) -> p u v j", u=2, v=2)[
                            64 * smp : 64 * smp + 64, :, v0
                        ]
                        eng = nc.sync if e < CFG["store_split"] else nc.gpsimd
                        eng.dma_start(yr2[e], sap)

            if rep_ctx is not None:
                rep_ctx.__exit__(None, None, None)

    nc.compile()
    return nc


_NC = None


def _get_program():
    global _NC
    if _NC is None:
        _NC = _build_program()
    return _NC


def kernel(x: np.ndarray) -> np.ndarray:
    assert x.shape == (B_FULL, 3, H, W), x.shape
    x = np.ascontiguousarray(x, dtype=np.float32)
    nc = _get_program()
    in_maps = []
    for c in range(N_CORES):
        in_maps.append(
            {
                "xs": x[c * B_CORE : (c + 1) * B_CORE],
                "wred": _WRED,
                "dftc": _DFTC,
                "wrep": _WREP,
            }
        )
    res = run_bass_kernel_spmd(nc, in_maps, core_ids=list(range(N_CORES)))
    out = np.concatenate([res.results[c]["ys"] for c in range(N_CORES)], axis=0)
    return out


# revision 31
# speedup vs baseline: 42811.0115x; 1.0152x over previous
"""Trainium2 Bass kernel for nn_DCTHighPass.

Reference computation (per sample, 512x512 RGB image):
  gray = 0.299 R + 0.587 G + 0.114 B
  tiles = 8x8 blocks of gray (64x64 tiles, row-major (ti, tj))
  mag = |fft2(tile)|
  (buggy mask touches only batch 3:6 / fft rows 3:6 -> never sampled below)
  img = mag tiles stacked into [4096*8, 8]
  out = bilinear_resize(img, 512, 512)

Key reduction: the height downsample (32768 -> 512, factor 64) samples only
input rows 64i+31 and 64i+32 with weight 0.5 each, i.e. fft-row 7 of tile
(ti=i//8, tj=8*(i%8)+3) and fft-row 0 of tile (ti, tj=8*(i%8)+4).  fft rows
0/7 of an 8x8 real tile need only three 8-weight row reductions of the tile
(plain sum, cos, sin), followed by an 8-point DFT along columns.  The width
upsample (8 -> 512) is a fixed [8,512] matrix.  So per output row i:
  v = 0.5*(|DFT(cos/sin rowsums of tile tj3)| + |DFT(colsum of tile tj4)|)
  out[i, :] = v @ W8
Only image columns 64p+24 .. 64p+39 (p = i%8) are ever used.

Pipeline per sample (per core, batch of 8 samples):
  stage1 (PE):  x[128-row chunk] as stationary operand (128 needed cols),
                weights = per-channel (gray-coef x {1,cos,sin} x row-group)
                -> PSUM [128=(p,cc), 192=(q,type,tI_l)]
  stage2 (PE):  4 block-diag DFT matmuls -> [64=(p,k), 192] cos/sin x g0/g1
  magnitude (ACT/DVE): sqrt of sum of squares, x0.5 -> V [64=(p,k), 64=tI]
  stage3 (PE):  V (stationary) @ p-masked replicated W8 -> [64=tI, 512]
  out rows i = 8*tI + p, interleaved stores back to HBM.
"""

import sys

sys.path.insert(0, "/opt/trn_rl_repo")

import math
import numpy as np

from concourse import bacc
import concourse.mybir as mybir
from concourse.tile import TileContext
from concourse.bass_utils import run_bass_kernel_spmd

N_CORES = 8
B_FULL = 64
B_CORE = B_FULL // N_CORES  # 8 samples per core
H = W = 512
K = 8  # fft tile size
NQ = 4  # 128-row chunks per image
DT = mybir.dt.float32


# ----------------------------------------------------------------------------
# host-side constants
# ----------------------------------------------------------------------------
def _make_constants():
    j = np.arange(K)
    cosr = np.cos(2 * np.pi * j / K)
    sinr = np.sin(2 * np.pi * j / K)

    # wred [128, 144]: free = 48*ch + 16*type + tI_l
    # type 0: plain sum (A), 1: cos rowsum (Cr), 2: sin rowsum (Ci)
    coef = [0.299, 0.587, 0.114]
    wtypes = [np.ones(K), cosr, sinr]
    wred = np.zeros((128, 4 * 48), dtype=np.float32)
    r = np.arange(128)
    for ch in range(3):
        for ty in range(3):
            for t in range(16):
                rows = slice(8 * t, 8 * t + 8)
                wred[rows, 48 * ch + 16 * ty + t] = coef[ch] * wtypes[ty]
    for ty in range(3):
        for t in range(16):
            rows = slice(8 * t, 8 * t + 8)
            wred[rows, 144 + 16 * ty + t] = wtypes[ty]

    # dft matrices C[v,c] = cos(2pi v c/8), S[v,c] = sin(2pi v c/8)
    v = np.arange(K)
    C8 = np.cos(2 * np.pi * np.outer(v, j) / K).astype(np.float32)
    S8 = np.sin(2 * np.pi * np.outer(v, j) / K).astype(np.float32)

    # dftc [128, 320]: 5 blocks of 64 cols: [C*g0 | S*g0 | -S*g0 | C*g1 | S*g1]
    # partition = 16p + cc (cc in 0..15, g = cc//8); out col = 64*s + 8p + k
    dftc = np.zeros((128, 320), dtype=np.float32)
    for p in range(8):
        for cc in range(16):
            g, c = divmod(cc, 8)
            for k in range(8):
                cv, sv = C8[k, c], S8[k, c]
                if g == 0:
                    dftc[16 * p + cc, 0 + 8 * p + k] = cv
                    dftc[16 * p + cc, 64 + 8 * p + k] = sv
                    dftc[16 * p + cc, 128 + 8 * p + k] = -sv
                else:
                    dftc[16 * p + cc, 192 + 8 * p + k] = cv
                    dftc[16 * p + cc, 256 + 8 * p + k] = sv

    # W8 [8, 512]: bilinear width resize 8 -> 512 (align_corners=False)
    src = (np.arange(W) + 0.5) * (K / W) - 0.5
    src = np.clip(src, 0.0, K - 1.0)
    i0 = np.floor(src).astype(np.int64)
    i1 = np.minimum(i0 + 1, K - 1)
    fr = (src - i0).astype(np.float32)
    W8 = np.zeros((K, W), dtype=np.float32)
    for jj in range(W):
        W8[i0[jj], jj] += 1.0 - fr[jj]
        W8[i1[jj], jj] += fr[jj]

    # wrep [64, 8*512]: block p holds W8 on partitions 8p..8p+7, zero elsewhere
    wrep = np.zeros((64, 8 * W), dtype=np.float32)
    for p in range(8):
        wrep[8 * p : 8 * p + 8, W * p : W * p + W] = W8

    return wred, dftc, wrep


_WRED, _DFTC, _WREP = _make_constants()


# ----------------------------------------------------------------------------
# bass program (identical on all cores; per-core inputs differ)
# ----------------------------------------------------------------------------
CFG = dict(ps1_bufs=3, psab_bufs=2, ps3_bufs=3, load_split=2, store_split=1,
           copy_pat="avva", xin_bufs=3, mid_bufs=2, wrep_pool=True,
           merge_stores=True, load_mode="sparse", loads_only=False, skip_stores=False,
           gray_pre=True)


def _build_program(repeat=1):
    nc = bacc.Bacc()

    xs = nc.declare_dram_parameter("xs", [B_CORE, 3, H, W], DT, isOutput=False)
    wred_d = nc.declare_dram_parameter("wred", [128, 192], DT, isOutput=False)
    dftc_d = nc.declare_dram_parameter("dftc", [128, 320], DT, isOutput=False)
    wrep_d = nc.declare_dram_parameter("wrep", [64, 8 * W], mybir.dt.float32r, isOutput=False)
    ys = nc.declare_dram_parameter("ys", [B_CORE, 1, H, W], DT, isOutput=True)

    with TileContext(nc) as tc:
        with (
            tc.tile_pool(name="consts", bufs=1) as cpool,
            tc.tile_pool(name="xin", bufs=CFG["xin_bufs"]) as xpool,
            tc.tile_pool(name="xa", bufs=2) as xapool,
            tc.tile_pool(name="mid", bufs=CFG["mid_bufs"]) as mpool,
            tc.tile_pool(name="outp", bufs=2) as opool,
            tc.tile_pool(name="ps1", bufs=CFG["ps1_bufs"], space="PSUM") as ps1pool,
            tc.tile_pool(name="ps2", bufs=CFG["psab_bufs"], space="PSUM") as ps2pool,
            tc.tile_pool(name="ps3", bufs=CFG["ps3_bufs"], space="PSUM") as ps3pool,
        ):
            wred_sb = cpool.tile([128, 192], DT, tag="wred")
            nc.sync.dma_start(wred_sb[:], wred_d[:])
            dftc_sb = cpool.tile([128, 320], DT, tag="dftc")
            nc.sync.dma_start(dftc_sb[:], dftc_d[:])
            wrep_sb = cpool.tile([64, 8 * W], mybir.dt.float32r, tag="wrep")
            (nc.gpsimd if CFG["wrep_pool"] else nc.sync).dma_start(
                wrep_sb[:], wrep_d[:]
            )

            rep_ctx = tc.For_i(0, repeat, 1) if repeat > 1 else None
            if rep_ctx is not None:
                rep_ctx.__enter__()
            for bg2 in range(B_CORE // 2):
                # ---- two samples per iteration: stage2/3 run at 128-wide ----
                xn = []
                li = 0
                for smp in range(2):
                    bg = 2 * bg2 + smp
                    if CFG["load_mode"] == "full":
                        # one contiguous 3MB DMA, then gather needed columns
                        xa = xapool.tile([128, 3 * NQ * W], DT, tag=f"xa{smp}")
                        nc.sync.dma_start(
                            xa.rearrange("p (ch q c) -> p ch q c", ch=3, q=NQ),
                            xs[bg].rearrange("ch (q p) c -> p ch q c", p=128),
                        )
                        xav = xa.rearrange(
                            "p (ch q g c) -> p ch q g c", ch=3, q=NQ, g=8
                        )
                        xneed = xpool.tile([128, 3 * NQ * 128], DT, tag=f"xn{smp}")
                        xneedv = xneed.rearrange(
                            "p (ch q g c) -> p ch q g c", ch=3, q=NQ, g=8
                        )
                        for ch in range(3):
                            eng = nc.vector if (ch + smp) % 2 == 0 else nc.scalar
                            if eng is nc.vector:
                                eng.tensor_copy(
                                    xneedv[:, ch], xav[:, ch, :, :, 24:40]
                                )
                            else:
                                eng.copy(xneedv[:, ch], xav[:, ch, :, :, 24:40])
                    else:
                        xneed = xpool.tile([128, 3 * NQ * 128], DT, tag=f"xn{smp}")
                        xneedv = xneed.rearrange(
                            "p (ch q g c) -> p ch q g c", ch=3, q=NQ, g=8
                        )
                        xsrc = xs[bg].rearrange(
                            "ch (q p) (g c) -> p ch q g c", p=128, g=8
                        )
                        for ch in range(3):
                            for q in range(NQ):
                                eng = (
                                    nc.scalar
                                    if (li % 6) < CFG["load_split"]
                                    else nc.sync
                                )
                                eng.dma_start(
                                    xneedv[:, ch, q], xsrc[:, ch, q, :, 24:40]
                                )
                                li += 1
                    xn.append(xneed.rearrange("p (blk c) -> p blk c", c=128))

                if CFG["loads_only"]:
                    continue
                # ---- stage 1: row reductions per sample ----
                rhs2 = mpool.tile([128, 2 * 192], DT, tag="rhs2")
                if CFG["gray_pre"]:
                    # gray = 0.299 R + 0.587 G + 0.114 B on ACT/DVE, then a
                    # single unscaled reduction matmul per (smp, q)
                    xgray = mpool.tile([128, 2 * 512], DT, tag="xgray")
                    for smp in range(2):
                        xg = xgray[:, 512 * smp : 512 * smp + 512]
                        xr = xn[smp].rearrange("p blk c -> p (blk c)")
                        t1 = mpool.tile([128, 512], DT, tag=f"t1{smp}")
                        nc.scalar.activation(
                            t1[:], xr[:, 512:1024],
                            mybir.ActivationFunctionType.Copy, scale=0.587,
                        )
                        t2 = mpool.tile([128, 512], DT, tag=f"t2{smp}")
                        nc.vector.scalar_tensor_tensor(
                            t2[:], xr[:, 0:512], 0.299, t1[:],
                            mybir.AluOpType.mult, mybir.AluOpType.add,
                        )
                        nc.vector.scalar_tensor_tensor(
                            xg, xr[:, 1024:1536], 0.114, t2[:],
                            mybir.AluOpType.mult, mybir.AluOpType.add,
                        )
                    xgv = xgray.rearrange("p (sq c) -> p sq c", c=128)
                    for smp in range(2):
                        ps1 = ps1pool.tile([128, 192], DT, tag="ps1")
                        for q in range(NQ):
                            nc.tensor.matmul(
                                ps1[:, 48 * q : 48 * q + 48],
                                xgv[:, 4 * smp + q],
                                wred_sb[:, 144:192],
                                start=True, stop=True,
                            )
                        nc.vector.tensor_copy(
                            rhs2[:, 192 * smp : 192 * smp + 192], ps1[:]
                        )
                else:
                    for smp in range(2):
                        ps1 = ps1pool.tile([128, 192], DT, tag="ps1")
                        for q in range(NQ):
                            for ch in range(3):
                                nc.tensor.matmul(
                                    ps1[:, 48 * q : 48 * q + 48],
                                    xn[smp][:, NQ * ch + q],
                                    wred_sb[:, 48 * ch : 48 * ch + 48],
                                    start=(ch == 0),
                                    stop=(ch == 2),
                                )
                        nc.vector.tensor_copy(
                            rhs2[:, 192 * smp : 192 * smp + 192], ps1[:]
                        )

                # ---- stage 2: DFT + height-blend fused via PSUM accumulation
                # psQ [64=(p,k), 512] = [R3 | I3 | R4 | I4] blocks of (smp, q, tI_l)
                rhs2v = rhs2.rearrange("p (s q blk) -> p s q blk", s=2, q=NQ)
                selA = rhs2v[:, :, :, 0:16]
                selCr = rhs2v[:, :, :, 16:32]
                selCi = rhs2v[:, :, :, 32:48]
                psQ = ps2pool.tile([64, 512], DT, tag="psQ")
                C0 = dftc_sb[:, 0:64]
                S0 = dftc_sb[:, 64:128]
                S0n = dftc_sb[:, 128:192]
                C1 = dftc_sb[:, 192:256]
                S1 = dftc_sb[:, 256:320]
                nc.tensor.matmul(psQ[:, 0:128], C0, selCr, start=True, stop=False)
                nc.tensor.matmul(psQ[:, 0:128], S0, selCi, start=False, stop=True)
                nc.tensor.matmul(psQ[:, 128:256], C0, selCi, start=True, stop=False)
                nc.tensor.matmul(psQ[:, 128:256], S0n, selCr, start=False, stop=True)
                nc.tensor.matmul(psQ[:, 256:384], C1, selA, start=True, stop=True)
                nc.tensor.matmul(psQ[:, 384:512], S1, selA, start=True, stop=True)

                # magnitudes: m = 0.5*sqrt(re^2 + im^2), [64, 128] each
                Sq = mybir.ActivationFunctionType.Square
                p3 = mpool.tile([64, 128], DT, tag="p3")
                nc.scalar.activation(p3[:], psQ[:, 0:128], Sq)
                q3 = mpool.tile([64, 128], DT, tag="q3")
                nc.scalar.activation(q3[:], psQ[:, 128:256], Sq)
                s3 = mpool.tile([64, 128], DT, tag="s3")
                nc.vector.tensor_add(s3[:], p3[:], q3[:])
                m3 = mpool.tile([64, 128], DT, tag="m3")
                nc.scalar.activation(
                    m3[:], s3[:], mybir.ActivationFunctionType.Sqrt, scale=0.25
                )
                p4 = mpool.tile([64, 128], DT, tag="p4")
                nc.scalar.activation(p4[:], psQ[:, 256:384], Sq)
                q4 = mpool.tile([64, 128], DT, tag="q4")
                nc.scalar.activation(q4[:], psQ[:, 384:512], Sq)
                s4 = mpool.tile([64, 128], DT, tag="s4")
                nc.vector.tensor_add(s4[:], p4[:], q4[:])
                m4 = mpool.tile([64, 128], DT, tag="m4")
                nc.scalar.activation(
                    m4[:], s4[:], mybir.ActivationFunctionType.Sqrt, scale=0.25
                )
                vt = mpool.tile([64, 128], mybir.dt.float32r, tag="vt")
                nc.vector.tensor_add(vt[:], m3[:], m4[:])

                # ---- stage 3: width resize; out partitions = (smp, tI) ----
                outse = opool.tile([128, NQ * W], DT, tag="outse")
                outso = opool.tile([128, NQ * W], DT, tag="outso")
                for p in range(8):
                    v, e2 = divmod(p, 2)
                    ps3 = ps3pool.tile([128, W], DT, tag="ps3")
                    nc.tensor.matmul(
                        ps3[:],
                        vt[:],
                        wrep_sb[:, W * p : W * p + W],
                        start=True, stop=True,
                    )
                    dst = (outso if e2 else outse)[:, W * v : W * v + W]
                    if CFG["copy_pat"][p % 4] == "v":
                        nc.vector.tensor_copy(dst, ps3[:])
                    else:
                        nc.scalar.copy(dst, ps3[:])

                # merged pair stores per sample: rows 8t+e and 8t+e+4
                for smp in range(2) if not CFG["skip_stores"] else []:
                    bg = 2 * bg2 + smp
                    yr2 = ys[bg, 0].rearrange(
                        "(t h e) j -> e t h j", h=2, e=4
                    )  # i = 8t + 4h + e
                    for e in range(4):
                        v0, e2 = divmod(e, 2)
                        src = outso if e2 else outse
                        sap = src.rearrange("p (u v j) -> p u v j", u=2, v=2)[
                            64 * smp : 64 * smp + 64, :, v0
                        ]
                        eng = nc.sync if e < CFG["store_split"] else nc.gpsimd
                        eng.dma_start(yr2[e], sap)

            if rep_ctx is not None:
                rep_ctx.__exit__(None, None, None)

    nc.compile()
    return nc


_NC = None


def _get_program():
    global _NC
    if _NC is None:
        _NC = _build_program()
    return _NC


def kernel(x: np.ndarray) -> np.ndarray:
    assert x.shape == (B_FULL, 3, H, W), x.shape
    x = np.ascontiguousarray(x, dtype=np.float32)
    nc = _get_program()
    in_maps = []
    for c in range(N_CORES):
        in_maps.append(
            {
                "xs": x[c * B_CORE : (c + 1) * B_CORE],
                "wred": _WRED,
                "dftc": _DFTC,
                "wrep": _WREP,
            }
        )
    res = run_bass_kernel_spmd(nc, in_maps, core_ids=list(range(N_CORES)))
    out = np.concatenate([res.results[c]["ys"] for c in range(N_CORES)], axis=0)
    return out


# revision 33
# speedup vs baseline: 43769.1077x; 1.0224x over previous
"""Trainium2 Bass kernel for nn_DCTHighPass.

Reference computation (per sample, 512x512 RGB image):
  gray = 0.299 R + 0.587 G + 0.114 B
  tiles = 8x8 blocks of gray (64x64 tiles, row-major (ti, tj))
  mag = |fft2(tile)|
  (buggy mask touches only batch 3:6 / fft rows 3:6 -> never sampled below)
  img = mag tiles stacked into [4096*8, 8]
  out = bilinear_resize(img, 512, 512)

Key reduction: the height downsample (32768 -> 512, factor 64) samples only
input rows 64i+31 and 64i+32 with weight 0.5 each, i.e. fft-row 7 of tile
(ti=i//8, tj=8*(i%8)+3) and fft-row 0 of tile (ti, tj=8*(i%8)+4).  fft rows
0/7 of an 8x8 real tile need only three 8-weight row reductions of the tile
(plain sum, cos, sin), followed by an 8-point DFT along columns.  The width
upsample (8 -> 512) is a fixed [8,512] matrix.  So per output row i:
  v = 0.5*(|DFT(cos/sin rowsums of tile tj3)| + |DFT(colsum of tile tj4)|)
  out[i, :] = v @ W8
Only image columns 64p+24 .. 64p+39 (p = i%8) are ever used.

Pipeline per sample (per core, batch of 8 samples):
  stage1 (PE):  x[128-row chunk] as stationary operand (128 needed cols),
                weights = per-channel (gray-coef x {1,cos,sin} x row-group)
                -> PSUM [128=(p,cc), 192=(q,type,tI_l)]
  stage2 (PE):  4 block-diag DFT matmuls -> [64=(p,k), 192] cos/sin x g0/g1
  magnitude (ACT/DVE): sqrt of sum of squares, x0.5 -> V [64=(p,k), 64=tI]
  stage3 (PE):  V (stationary) @ p-masked replicated W8 -> [64=tI, 512]
  out rows i = 8*tI + p, interleaved stores back to HBM.
"""

import sys

sys.path.insert(0, "/opt/trn_rl_repo")

import math
import numpy as np

from concourse import bacc
import concourse.mybir as mybir
from concourse.tile import TileContext
from concourse.bass_utils import run_bass_kernel_spmd

N_CORES = 8
B_FULL = 64
B_CORE = B_FULL // N_CORES  # 8 samples per core
H = W = 512
K = 8  # fft tile size
NQ = 4  # 128-row chunks per image
DT = mybir.dt.float32


# ----------------------------------------------------------------------------
# host-side constants
# ----------------------------------------------------------------------------
def _make_constants():
    j = np.arange(K)
    cosr = np.cos(2 * np.pi * j / K)
    sinr = np.sin(2 * np.pi * j / K)

    # wred [128, 144]: free = 48*ch + 16*type + tI_l
    # type 0: plain sum (A), 1: cos rowsum (Cr), 2: sin rowsum (Ci)
    coef = [0.299, 0.587, 0.114]
    wtypes = [np.ones(K), cosr, sinr]
    wred = np.zeros((128, 4 * 48), dtype=np.float32)
    r = np.arange(128)
    for ch in range(3):
        for ty in range(3):
            for t in range(16):
                rows = slice(8 * t, 8 * t + 8)
                wred[rows, 48 * ch + 16 * ty + t] = coef[ch] * wtypes[ty]
    for ty in range(3):
        for t in range(16):
            rows = slice(8 * t, 8 * t + 8)
            wred[rows, 144 + 16 * ty + t] = wtypes[ty]

    # dft matrices C[v,c] = cos(2pi v c/8), S[v,c] = sin(2pi v c/8)
    v = np.arange(K)
    C8 = np.cos(2 * np.pi * np.outer(v, j) / K).astype(np.float32)
    S8 = np.sin(2 * np.pi * np.outer(v, j) / K).astype(np.float32)

    # dftc [128, 320]: 5 blocks of 64 cols: [C*g0 | S*g0 | -S*g0 | C*g1 | S*g1]
    # partition = 16p + cc (cc in 0..15, g = cc//8); out col = 64*s + 8p + k
    dftc = np.zeros((128, 320), dtype=np.float32)
    for p in range(8):
        for cc in range(16):
            g, c = divmod(cc, 8)
            for k in range(8):
                cv, sv = C8[k, c], S8[k, c]
                if g == 0:
                    dftc[16 * p + cc, 0 + 8 * p + k] = cv
                    dftc[16 * p + cc, 64 + 8 * p + k] = sv
                    dftc[16 * p + cc, 128 + 8 * p + k] = -sv
                else:
                    dftc[16 * p + cc, 192 + 8 * p + k] = cv
                    dftc[16 * p + cc, 256 + 8 * p + k] = sv

    # W8 [8, 512]: bilinear width resize 8 -> 512 (align_corners=False)
    src = (np.arange(W) + 0.5) * (K / W) - 0.5
    src = np.clip(src, 0.0, K - 1.0)
    i0 = np.floor(src).astype(np.int64)
    i1 = np.minimum(i0 + 1, K - 1)
    fr = (src - i0).astype(np.float32)
    W8 = np.zeros((K, W), dtype=np.float32)
    for jj in range(W):
        W8[i0[jj], jj] += 1.0 - fr[jj]
        W8[i1[jj], jj] += fr[jj]

    # wrep [64, 8*512]: block p holds W8 on partitions 8p..8p+7, zero elsewhere
    wrep = np.zeros((64, 8 * W), dtype=np.float32)
    for p in range(8):
        wrep[8 * p : 8 * p + 8, W * p : W * p + W] = W8

    return wred, dftc, wrep


_WRED, _DFTC, _WREP = _make_constants()


# ----------------------------------------------------------------------------
# bass program (identical on all cores; per-core inputs differ)
# ----------------------------------------------------------------------------
CFG = dict(ps1_bufs=3, psab_bufs=2, ps3_bufs=3, load_split=2, store_split=1,
           copy_pat="avva", xin_bufs=3, mid_bufs=2, wrep_pool=True,
           merge_stores=True, load_mode="sparse", loads_only=False, skip_stores=False,
           gray_pre=True, wide_ps3=False)


def _build_program(repeat=1):
    nc = bacc.Bacc()

    xs = nc.declare_dram_parameter("xs", [B_CORE, 3, H, W], DT, isOutput=False)
    wred_d = nc.declare_dram_parameter("wred", [128, 192], DT, isOutput=False)
    dftc_d = nc.declare_dram_parameter("dftc", [128, 320], DT, isOutput=False)
    wrep_d = nc.declare_dram_parameter("wrep", [64, 8 * W], mybir.dt.float32r, isOutput=False)
    ys = nc.declare_dram_parameter("ys", [B_CORE, 1, H, W], DT, isOutput=True)

    with TileContext(nc) as tc:
        with (
            tc.tile_pool(name="consts", bufs=1) as cpool,
            tc.tile_pool(name="xin", bufs=CFG["xin_bufs"]) as xpool,
            tc.tile_pool(name="xa", bufs=2) as xapool,
            tc.tile_pool(name="mid", bufs=CFG["mid_bufs"]) as mpool,
            tc.tile_pool(name="outp", bufs=CFG.get("outp_bufs", 2)) as opool,
            tc.tile_pool(name="ps1", bufs=CFG["ps1_bufs"], space="PSUM") as ps1pool,
            tc.tile_pool(name="ps2", bufs=CFG["psab_bufs"], space="PSUM") as ps2pool,
            tc.tile_pool(name="ps3", bufs=CFG["ps3_bufs"], space="PSUM") as ps3pool,
        ):
            wred_sb = cpool.tile([128, 192], DT, tag="wred")
            nc.sync.dma_start(wred_sb[:], wred_d[:])
            dftc_sb = cpool.tile([128, 320], DT, tag="dftc")
            nc.sync.dma_start(dftc_sb[:], dftc_d[:])
            wrep_sb = cpool.tile([64, 8 * W], mybir.dt.float32r, tag="wrep")
            (nc.gpsimd if CFG["wrep_pool"] else nc.sync).dma_start(
                wrep_sb[:], wrep_d[:]
            )

            rep_ctx = tc.For_i(0, repeat, 1) if repeat > 1 else None
            if rep_ctx is not None:
                rep_ctx.__enter__()
            for bg2 in range(B_CORE // 2):
                # ---- two samples per iteration: stage2/3 run at 128-wide ----
                xn = []
                li = 0
                for smp in range(2):
                    bg = 2 * bg2 + smp
                    if CFG["load_mode"] == "full":
                        # one contiguous 3MB DMA, then gather needed columns
                        xa = xapool.tile([128, 3 * NQ * W], DT, tag=f"xa{smp}")
                        nc.sync.dma_start(
                            xa.rearrange("p (ch q c) -> p ch q c", ch=3, q=NQ),
                            xs[bg].rearrange("ch (q p) c -> p ch q c", p=128),
                        )
                        xav = xa.rearrange(
                            "p (ch q g c) -> p ch q g c", ch=3, q=NQ, g=8
                        )
                        xneed = xpool.tile([128, 3 * NQ * 128], DT, tag=f"xn{smp}")
                        xneedv = xneed.rearrange(
                            "p (ch q g c) -> p ch q g c", ch=3, q=NQ, g=8
                        )
                        for ch in range(3):
                            eng = nc.vector if (ch + smp) % 2 == 0 else nc.scalar
                            if eng is nc.vector:
                                eng.tensor_copy(
                                    xneedv[:, ch], xav[:, ch, :, :, 24:40]
                                )
                            else:
                                eng.copy(xneedv[:, ch], xav[:, ch, :, :, 24:40])
                    else:
                        xneed = xpool.tile([128, 3 * NQ * 128], DT, tag=f"xn{smp}")
                        xneedv = xneed.rearrange(
                            "p (ch q g c) -> p ch q g c", ch=3, q=NQ, g=8
                        )
                        xsrc = xs[bg].rearrange(
                            "ch (q p) (g c) -> p ch q g c", p=128, g=8
                        )
                        for ch in range(3):
                            for q in range(NQ):
                                eng = (
                                    nc.scalar
                                    if (li % 6) < CFG["load_split"]
                                    else nc.sync
                                )
                                eng.dma_start(
                                    xneedv[:, ch, q], xsrc[:, ch, q, :, 24:40]
                                )
                                li += 1
                    xn.append(xneed.rearrange("p (blk c) -> p blk c", c=128))

                if CFG["loads_only"]:
                    continue
                # ---- stage 1: row reductions per sample ----
                rhs2 = mpool.tile([128, 2 * 192], DT, tag="rhs2")
                if CFG["gray_pre"]:
                    # gray = 0.299 R + 0.587 G + 0.114 B on ACT/DVE, then a
                    # single unscaled reduction matmul per (smp, q)
                    xgray = mpool.tile([128, 2 * 512], DT, tag="xgray")
                    for smp in range(2):
                        xg = xgray[:, 512 * smp : 512 * smp + 512]
                        xr = xn[smp].rearrange("p blk c -> p (blk c)")
                        t1 = mpool.tile([128, 512], DT, tag=f"t1{smp}")
                        nc.scalar.activation(
                            t1[:], xr[:, 512:1024],
                            mybir.ActivationFunctionType.Copy, scale=0.587,
                        )
                        t2 = mpool.tile([128, 512], DT, tag=f"t2{smp}")
                        nc.vector.scalar_tensor_tensor(
                            t2[:], xr[:, 0:512], 0.299, t1[:],
                            mybir.AluOpType.mult, mybir.AluOpType.add,
                        )
                        nc.vector.scalar_tensor_tensor(
                            xg, xr[:, 1024:1536], 0.114, t2[:],
                            mybir.AluOpType.mult, mybir.AluOpType.add,
                        )
                    xgv = xgray.rearrange("p (sq c) -> p sq c", c=128)
                    for smp in range(2):
                        ps1 = ps1pool.tile([128, 192], DT, tag="ps1")
                        for q in range(NQ):
                            nc.tensor.matmul(
                                ps1[:, 48 * q : 48 * q + 48],
                                xgv[:, 4 * smp + q],
                                wred_sb[:, 144:192],
                                start=True, stop=True,
                            )
                        nc.vector.tensor_copy(
                            rhs2[:, 192 * smp : 192 * smp + 192], ps1[:]
                        )
                else:
                    for smp in range(2):
                        ps1 = ps1pool.tile([128, 192], DT, tag="ps1")
                        for q in range(NQ):
                            for ch in range(3):
                                nc.tensor.matmul(
                                    ps1[:, 48 * q : 48 * q + 48],
                                    xn[smp][:, NQ * ch + q],
                                    wred_sb[:, 48 * ch : 48 * ch + 48],
                                    start=(ch == 0),
                                    stop=(ch == 2),
                                )
                        nc.vector.tensor_copy(
                            rhs2[:, 192 * smp : 192 * smp + 192], ps1[:]
                        )

                # ---- stage 2: DFT + height-blend fused via PSUM accumulation
                # psQ [64=(p,k), 512] = [R3 | I3 | R4 | I4] blocks of (smp, q, tI_l)
                rhs2v = rhs2.rearrange("p (s q blk) -> p s q blk", s=2, q=NQ)
                selA = rhs2v[:, :, :, 0:16]
                selCr = rhs2v[:, :, :, 16:32]
                selCi = rhs2v[:, :, :, 32:48]
                psQ = ps2pool.tile([64, 512], DT, tag="psQ")
                C0 = dftc_sb[:, 0:64]
                S0 = dftc_sb[:, 64:128]
                S0n = dftc_sb[:, 128:192]
                C1 = dftc_sb[:, 192:256]
                S1 = dftc_sb[:, 256:320]
                nc.tensor.matmul(psQ[:, 0:128], C0, selCr, start=True, stop=False)
                nc.tensor.matmul(psQ[:, 0:128], S0, selCi, start=False, stop=True)
                nc.tensor.matmul(psQ[:, 128:256], C0, selCi, start=True, stop=False)
                nc.tensor.matmul(psQ[:, 128:256], S0n, selCr, start=False, stop=True)
                nc.tensor.matmul(psQ[:, 256:384], C1, selA, start=True, stop=True)
                nc.tensor.matmul(psQ[:, 384:512], S1, selA, start=True, stop=True)

                # magnitudes: m = 0.5*sqrt(re^2 + im^2), [64, 128] each
                Sq = mybir.ActivationFunctionType.Square
                p3 = mpool.tile([64, 128], DT, tag="p3")
                nc.scalar.activation(p3[:], psQ[:, 0:128], Sq)
                q3 = mpool.tile([64, 128], DT, tag="q3")
                nc.scalar.activation(q3[:], psQ[:, 128:256], Sq)
                s3 = mpool.tile([64, 128], DT, tag="s3")
                nc.vector.tensor_add(s3[:], p3[:], q3[:])
                m3 = mpool.tile([64, 128], DT, tag="m3")
                nc.scalar.activation(
                    m3[:], s3[:], mybir.ActivationFunctionType.Sqrt, scale=0.25
                )
                p4 = mpool.tile([64, 128], DT, tag="p4")
                nc.scalar.activation(p4[:], psQ[:, 256:384], Sq)
                q4 = mpool.tile([64, 128], DT, tag="q4")
                nc.scalar.activation(q4[:], psQ[:, 384:512], Sq)
                s4 = mpool.tile([64, 128], DT, tag="s4")
                nc.vector.tensor_add(s4[:], p4[:], q4[:])
                m4 = mpool.tile([64, 128], DT, tag="m4")
                nc.scalar.activation(
                    m4[:], s4[:], mybir.ActivationFunctionType.Sqrt, scale=0.25
                )
                vt = mpool.tile([64, 128], mybir.dt.float32r, tag="vt")
                nc.vector.tensor_add(vt[:], m3[:], m4[:])

                # ---- stage 3: width resize; out partitions = (smp, tI) ----
                outse = opool.tile([128, NQ * W], DT, tag="outse")
                outso = opool.tile([128, NQ * W], DT, tag="outso")
                if CFG["wide_ps3"]:
                    # two stage-3 matmuls share one 2-bank PSUM tile; one
                    # [128,1024] copy drains both (half the copy instructions)
                    for pp in range(4):
                        ps3w = ps3pool.tile([128, 2 * W], DT, tag="ps3w")
                        for half in range(2):
                            p = 2 * pp + half
                            nc.tensor.matmul(
                                ps3w[:, W * half : W * half + W],
                                vt[:],
                                wrep_sb[:, W * p : W * p + W],
                                start=True, stop=True,
                            )
                        # p=2pp -> even (outse, v=pp), p=2pp+1 -> odd (outso, v=pp)
                        dste = outse[:, W * pp : W * pp + W]
                        dsto = outso[:, W * pp : W * pp + W]
                        if CFG["copy_pat"][pp % 4] == "v":
                            nc.vector.tensor_copy(dste, ps3w[:, 0:W])
                            nc.scalar.copy(dsto, ps3w[:, W : 2 * W])
                        else:
                            nc.scalar.copy(dste, ps3w[:, 0:W])
                            nc.vector.tensor_copy(dsto, ps3w[:, W : 2 * W])
                else:
                    for p in range(8):
                        v, e2 = divmod(p, 2)
                        ps3 = ps3pool.tile([128, W], DT, tag="ps3")
                        nc.tensor.matmul(
                            ps3[:],
                            vt[:],
                            wrep_sb[:, W * p : W * p + W],
                            start=True, stop=True,
                        )
                        dst = (outso if e2 else outse)[:, W * v : W * v + W]
                        if CFG["copy_pat"][p % 4] == "v":
                            nc.vector.tensor_copy(dst, ps3[:])
                        else:
                            nc.scalar.copy(dst, ps3[:])

                # merged pair stores per sample: rows 8t+e and 8t+e+4
                for smp in range(2) if not CFG["skip_stores"] else []:
                    bg = 2 * bg2 + smp
                    yr2 = ys[bg, 0].rearrange(
                        "(t h e) j -> e t h j", h=2, e=4
                    )  # i = 8t + 4h + e
                    for e in range(4):
                        v0, e2 = divmod(e, 2)
                        src = outso if e2 else outse
                        sap = src.rearrange("p (u v j) -> p u v j", u=2, v=2)[
                            64 * smp : 64 * smp + 64, :, v0
                        ]
                        eng = nc.sync if e < CFG["store_split"] else nc.gpsimd
                        eng.dma_start(yr2[e], sap)

            if rep_ctx is not None:
                rep_ctx.__exit__(None, None, None)

    nc.compile()
    return nc


_NC = None


def _get_program():
    global _NC
    if _NC is None:
        _NC = _build_program()
    return _NC


def kernel(x: np.ndarray) -> np.ndarray:
    assert x.shape == (B_FULL, 3, H, W), x.shape
    x = np.ascontiguousarray(x, dtype=np.float32)
    nc = _get_program()
    in_maps = []
    for c in range(N_CORES):
        in_maps.append(
            {
                "xs": x[c * B_CORE : (c + 1) * B_CORE],
                "wred": _WRED,
                "dftc": _DFTC,
                "wrep": _WREP,
            }
        )
    res = run_bass_kernel_spmd(nc, in_maps, core_ids=list(range(N_CORES)))
    out = np.concatenate([res.results[c]["ys"] for c in range(N_CORES)], axis=0)
    return out
